# revision 1
# baseline (speedup 1.0000x reference)
"""DensityAwareChamferLoss Trainium2 kernel.

Strategy: 8 cores = (4 batches) x (2 NN directions). Each core runs an
identical SPMD program computing, for 8192 query points against 8192
candidate points, the argmin of squared euclidean distance:

  PE:  s = 2*q.c - |c|^2 at fp32-grade precision but bf16 matmul speed
       (1 cy/row): error-compensated bf16^3 decomposition packed along
       the contraction dim as ONE K=21 bf16 matmul per tile — product
       terms (qh,Ch)(qh,Cl)(ql,Ch)(ql,Cl)(qh,Cm)(qm,Ch) with C=2c plus
       three |c|^2 rows; exact bf16 products accumulate in fp32 PSUM,
       residual ~2^-24 (plain fp32 matmul is 4 cy/row = 874us/core;
       fp32r is tf32-grade and flips ~1.8% of argmins).
  ACT: d = |q|^2 - s  (scale=-1, per-partition bias), cast bf16 -> SBUF
  DVE: fused min-reduce over the [128, 8192] strip (tensor_scalar accum),
       then max_index to recover up to 8 positions matching the min.
       (A tensor_tensor_reduce fold variant — fold=True, ~574us modeled,
       CoreSim-exact — is present but disabled: its only HW attempt hit
       NRT_EXEC_UNIT_UNRECOVERABLE and could not be re-verified.)

Host: bf16 ties (~0.4% of rows) are resolved by recomputing that row's
distances in fp32 (reproduces the reference argmin: 0 flips measured in
numpy simulation and CoreSim); counts/weights/loss are O(N) numpy.

Engine budget per core (HW-calibrated cost model): PE ~250us, ACT
~580us, DVE ~690us (bound: max_index at 1x). With strip_bufs=4 /
small_bufs=12 the cross-tile pipeline hides everything but DVE:
~715us total — the config verified on silicon (PASS, rel err 7.2e-8).
"""

import sys

if "/opt/trn_rl_repo" not in sys.path:
    sys.path.insert(0, "/opt/trn_rl_repo")

import numpy as np

B = 4
N = 8192
QT = N // 128  # query tiles per core
N_CORES = 8

_CACHE = {}


def _build(mm_dtype="float32", do_accum=True, do_argidx=True, reps=1,
           strip_bufs=2, psum_bufs=2, small_bufs=4, kdim=4, fold=False):
    from contextlib import ExitStack

    import concourse.bacc as bacc
    import concourse.bass as bass
    import concourse.tile as tile
    from concourse import mybir

    f32 = mybir.dt.float32
    mmdt = getattr(mybir.dt, mm_dtype)
    bf16 = mybir.dt.bfloat16
    u32 = mybir.dt.uint32

    if kdim != 4:
        mmdt = bf16
    nc = bacc.Bacc("TRN2", target_bir_lowering=False, debug=False)
    qt4 = nc.dram_tensor("qt4", [kdim, N], mmdt, kind="ExternalInput")
    ct4 = nc.dram_tensor("ct4", [kdim, N], mmdt, kind="ExternalInput")
    qsq = nc.dram_tensor("qsq", [128, QT], f32, kind="ExternalInput")
    if do_argidx:
        out_idx = nc.dram_tensor("out_idx", [QT, 128, 8], u32, kind="ExternalOutput")
    else:
        out_min = nc.dram_tensor("out_min", [QT, 128, 8], f32, kind="ExternalOutput")

    with tile.TileContext(nc) as tc:
        with ExitStack() as ctx:
            const = ctx.enter_context(tc.tile_pool(name="const", bufs=1))
            strips = ctx.enter_context(tc.tile_pool(name="strip", bufs=strip_bufs))
            psum = ctx.enter_context(
                tc.tile_pool(name="psum", bufs=psum_bufs, space="PSUM"))
            small = ctx.enter_context(tc.tile_pool(name="small", bufs=small_bufs))

            qt4_s = const.tile([kdim, N], mmdt)
            nc.sync.dma_start(qt4_s[:], qt4.ap())
            ct4_s = const.tile([kdim, N], mmdt)
            nc.sync.dma_start(ct4_s[:], ct4.ap())
            qsq_s = const.tile([128, QT], f32)
            nc.sync.dma_start(qsq_s[:], qsq.ap())
            zeros8 = const.tile([128, 8], f32)
            nc.vector.memset(zeros8[:], 0.0)

            for t in [tt for _ in range(reps) for tt in range(QT)]:
                strip = strips.tile([128, N], bf16, tag="strip")
                for g in range(4):
                    ps = psum.tile([128, 2048], f32, tag="ps")
                    for j in range(4):
                        nc.tensor.matmul(
                            ps[:, j * 512 : (j + 1) * 512],
                            qt4_s[:, t * 128 : (t + 1) * 128],
                            ct4_s[:, g * 2048 + j * 512 : g * 2048 + (j + 1) * 512],
                            start=True,
                            stop=True,
                        )
                    # d = -s + |q|^2, cast to bf16
                    nc.scalar.activation(
                        strip[:, g * 2048 : (g + 1) * 2048],
                        ps[:],
                        mybir.ActivationFunctionType.Identity,
                        bias=qsq_s[:, t : t + 1],
                        scale=-1.0,
                    )
                if fold == "safe":
                    # same fold using only HW-verified encodings: plain TT min
                    # (bf16 2x) then the proven tensor_scalar accum on h
                    dmin = small.tile([128, 1], f32, tag="dmin")
                    h = small.tile([128, N // 2], bf16, tag="h")
                    nc.vector.tensor_tensor(
                        out=h[:],
                        in0=strip[:, : N // 2],
                        in1=strip[:, N // 2 :],
                        op=mybir.AluOpType.min,
                    )
                    nc.vector.tensor_scalar(
                        out=h[:],
                        in0=h[:],
                        scalar1=0.0,
                        scalar2=None,
                        op0=mybir.AluOpType.add,
                        op1=mybir.AluOpType.min,
                        accum_out=dmin[:],
                    )
                elif fold:
                    # fused: h = min(lo half, hi half) AND dmin = min(h)
                    dmin = small.tile([128, 1], f32, tag="dmin")
                    h = small.tile([128, N // 2], bf16, tag="h")
                    nc.vector.tensor_tensor_reduce(
                        out=h[:],
                        in0=strip[:, : N // 2],
                        in1=strip[:, N // 2 :],
                        scale=1.0,
                        scalar=3.0e38,
                        op0=mybir.AluOpType.min,
                        op1=mybir.AluOpType.min,
                        accum_out=dmin[:],
                    )
                elif do_accum:
                    # fused: rewrite strip in place (x+0) and min-reduce into dmin
                    dmin = small.tile([128, 1], f32, tag="dmin")
                    nc.vector.tensor_scalar(
                        out=strip[:],
                        in0=strip[:],
                        scalar1=0.0,
                        scalar2=None,
                        op0=mybir.AluOpType.add,
                        op1=mybir.AluOpType.min,
                        accum_out=dmin[:],
                    )
                if do_argidx:
                    # broadcast dmin to [128, 8] bf16 via ACT (scale=0, bias=dmin)
                    min8 = small.tile([128, 8], bf16, tag="min8")
                    nc.scalar.activation(
                        min8[:],
                        zeros8[:],
                        mybir.ActivationFunctionType.Identity,
                        bias=dmin[:],
                        scale=0.0,
                    )
                    idx8 = small.tile([128, 8], u32, tag="idx8")
                    nc.vector.max_index(idx8[:], min8[:],
                                        h[:] if fold else strip[:])
                    nc.sync.dma_start(out_idx.ap()[t], idx8[:])
                elif do_accum:
                    omin = small.tile([128, 8], f32, tag="omin")
                    nc.scalar.activation(
                        omin[:], zeros8[:],
                        mybir.ActivationFunctionType.Identity,
                        bias=dmin[:], scale=0.0,
                    )
                    nc.sync.dma_start(out_min.ap()[t], omin[:])
                else:
                    probe = small.tile([128, 8], f32, tag="omin")
                    sap = bass.AP(strip[:].tensor, strip[:].offset,
                                  [strip[:].ap[0], [1024, 8]])
                    nc.vector.tensor_copy(probe[:], sap)
                    nc.sync.dma_start(out_min.ap()[t], probe[:])

    nc.compile()
    return nc


def _prep_core_inputs(q, c):
    # q, c: [N, 3] float32
    qt4 = np.empty((4, N), np.float32)
    qt4[0:3] = q.T
    qt4[3] = 1.0
    ct4 = np.empty((4, N), np.float32)
    ct4[0:3] = 2.0 * c.T
    csq = np.sum(c.astype(np.float32) * c.astype(np.float32), axis=1)
    ct4[3] = -csq
    qsq_flat = np.sum(q.astype(np.float32) * q.astype(np.float32), axis=1)
    qsq = qsq_flat.reshape(QT, 128).T.copy()
    return {"qt4": qt4, "ct4": ct4, "qsq": qsq}


def _bf16_split3(x):
    # x (fp32) == hi + lo + mid to ~2^-24 rel; parts exactly bf16
    import ml_dtypes

    bf = ml_dtypes.bfloat16
    hi = x.astype(bf)
    r1 = (x - hi.astype(np.float32)).astype(np.float32)
    lo = r1.astype(bf)
    r2 = (r1 - lo.astype(np.float32)).astype(np.float32)
    mid = r2.astype(bf)
    return hi, lo, mid


def _prep_core_inputs_k21(q, c):
    """Error-compensated bf16^3 decomposition packed along K=21.

    s = sum_k lhsT[k]*rhs[k] = 2q.c - |c|^2 to ~2^-24 relative:
    product terms (qh,Ch),(qh,Cl),(ql,Ch),(ql,Cl),(qh,Cm),(qm,Ch) where
    C = 2c, plus (1,-csq_{h,l,m}). Each part is exactly bf16; PE computes
    exact bf16 x bf16 products accumulated in fp32 PSUM.
    """
    import ml_dtypes

    bf = ml_dtypes.bfloat16
    qh, ql, qm = _bf16_split3(np.ascontiguousarray(q.T, np.float32))  # [3, N]
    Ch, Cl, Cm = _bf16_split3(2.0 * np.ascontiguousarray(c.T, np.float32))
    csq = np.sum(c.astype(np.float32) * c.astype(np.float32), axis=1)
    sh, sl, sm = _bf16_split3(-csq)
    ones = np.ones((1, N), bf)
    qt = np.concatenate(
        [qh, qh, ql, ql, qh, qm, ones, ones, ones], axis=0
    ).astype(bf)
    ct = np.concatenate(
        [Ch, Cl, Ch, Cl, Cm, Ch, sh[None], sl[None], sm[None]], axis=0
    ).astype(bf)
    qsq_flat = np.sum(q.astype(np.float32) * q.astype(np.float32), axis=1)
    qsq = qsq_flat.reshape(QT, 128).T.copy()
    return {"qt4": qt, "ct4": ct, "qsq": qsq}


def _d_row_fp32(q_row, c_all):
    # reference-formula distances of one query row vs all candidates, fp32
    return (
        np.sum(q_row * q_row).astype(np.float32)
        + np.sum(c_all * c_all, axis=1)
        - 2.0 * (c_all @ q_row)
    ).astype(np.float32)


def _indices_from_out(idx8, q, c):
    # idx8: [QT, 128, 8] uint32 -> idx [N] with host tie fixup
    cand = idx8.reshape(N, 8)
    idx = cand[:, 0].astype(np.int64)
    ambiguous = np.where(cand[:, 1] != np.uint32(0xFFFFFFFF))[0]
    for r in ambiguous:
        d = _d_row_fp32(q[r], c)
        idx[r] = int(np.argmin(d))
    return idx


def _loss_one(q, c, idx):
    # mean(1 - exp(-d) * (1/(count+eps))) for one direction (frac terms = 1)
    d = np.sum((q - c[idx]) ** 2, axis=1).astype(np.float32)
    cnt = np.bincount(idx, minlength=N).astype(np.float32)
    w = np.float32(1.0) / (cnt[idx] + np.float32(1e-6))
    return np.mean(np.float32(1.0) - np.exp(-d) * w, dtype=np.float32)


def run_cores(in_maps, trace=False):
    from concourse.bass_utils import run_bass_kernel_spmd

    if "nc" not in _CACHE:
        # fold=True models ~574us and is CoreSim-exact, but it crashes the
        # exec unit on silicon (NRT_EXEC_UNIT_UNRECOVERABLE, reproduced 2/2
        # on a freshly-verified-healthy device) — the tensor_tensor_reduce
        # bf16-out+accum encoding is the suspect. Keep it disabled.
        _CACHE["nc"] = _build(kdim=21, strip_bufs=4, small_bufs=12)
    nc = _CACHE["nc"]
    res = run_bass_kernel_spmd(
        nc, in_maps, core_ids=list(range(N_CORES)), trace=trace
    )
    return res


def kernel(gts, preds):
    gts = np.ascontiguousarray(np.asarray(gts, dtype=np.float32))
    preds = np.ascontiguousarray(np.asarray(preds, dtype=np.float32))

    qc = []  # per-core (q, c)
    for core in range(N_CORES):
        b, direction = core >> 1, core & 1
        if direction == 0:
            qc.append((gts[b], preds[b]))
        else:
            qc.append((preds[b], gts[b]))

    in_maps = [_prep_core_inputs_k21(q, c) for (q, c) in qc]
    res = run_cores(in_maps)

    loss = np.zeros(B, np.float32)
    per_dir = {}
    for core in range(N_CORES):
        q, c = qc[core]
        idx = _indices_from_out(np.asarray(res.results[core]["out_idx"]), q, c)
        per_dir[core] = _loss_one(q, c, idx)
    for b in range(B):
        loss[b] = (per_dir[2 * b] + per_dir[2 * b + 1]) / np.float32(2.0)
    return loss


def _indices_from_out_fold(idx8, q, c):
    # idx8: [QT, 128, 8] positions in the folded half-strip; each expands to
    # {p, p+N/2}. Pick by exact fp32 reference-formula distance; full-row
    # fixup on exact ties or candidate-list overflow.
    H = N // 2
    cand_h = idx8.reshape(N, 8)
    valid = cand_h != np.uint32(0xFFFFFFFF)
    ch = np.where(valid, cand_h, 0).astype(np.int64)
    cands = np.concatenate([ch, ch + H], axis=1)  # [N, 16]
    vmask = np.concatenate([valid, valid], axis=1)
    qsq = np.sum(q.astype(np.float32) * q, axis=1).astype(np.float32)
    csq = np.sum(c.astype(np.float32) * c, axis=1).astype(np.float32)
    dots = np.einsum("rkd,rd->rk", c[cands], q.astype(np.float32),
                     dtype=np.float32).astype(np.float32)
    dc = (qsq[:, None] + csq[cands] - np.float32(2.0) * dots).astype(np.float32)
    dc[~vmask] = np.inf
    best = np.argmin(dc, axis=1)
    idx = cands[np.arange(N), best]
    dmin = dc[np.arange(N), best]
    n_min = (dc == dmin[:, None]).sum(1)
    fix = np.where((n_min > 1) | valid[:, 7])[0]
    for r in fix:
        idx[r] = int(np.argmin(_d_row_fp32(q[r], c)))
    return idx



# revision 3
# speedup vs baseline: 1.7221x; 1.7221x over previous
"""DensityAwareChamferLoss Trainium2 kernel — fold-pipeline version.

8 cores = (4 batches) x (2 NN directions), SPMD. Each core finds, for
8192 query points, the argmax over 8192 candidates of
s = 2*q.c - |c|^2 (argmax s == argmin squared distance; the per-row
|q|^2 bias is constant and dropped, so no ACT bias pass is needed).

Per 128-query tile (64 tiles/core):
  PE:   16 matmuls (K=21 error-compensated bf16^3 decomposition) into
        8 PSUM sub-groups of [128,1024] fp32 (pool bufs=4).
  DVE:  evicts 4 sub-groups as 2 pair-TTs max(Q2g,Q2g+1) -> bf16
        (cost = output size: 2 el/cy effective on fp32 PSUM).
  ACT:  evicts the other 4 as Identity casts fp32->bf16.
  Pool: 3 bf16 merge-TTs (gpsimd) fold the 6 arrays toward one.
  DVE:  2 merge-TTs + 2 halving folds -> H3[128,256] bf16, then
        tensor_scalar max-accum (smax) + max_index (up to 8 matching
        folded positions, ascending).
Host: each folded position p expands to candidates {p + 256k, k<32};
exact fp32 rescore picks the reference argmin (monotone bf16 rounding
guarantees the true argmax's folded slot matches smax); exact ties or
8-slot overflow fall back to a full-row fp32 recompute. Counts/loss are
O(N) numpy as before.

Engine budget per core (cost model): PE ~220us, DVE ~306us, ACT ~300us,
Pool ~291us vs baseline's DVE ~690us.
"""

import sys

if "/opt/trn_rl_repo" not in sys.path:
    sys.path.insert(0, "/opt/trn_rl_repo")

import numpy as np

B = 4
N = 8192
QT = N // 128
N_CORES = 8
KDIM = 21
MFOLD = 256          # final folded width scanned by max_index
EXPAND = N // MFOLD  # 32 candidates per folded position

_CACHE = {}


def _build_fold(big_bufs=3, small_bufs=6, psum_bufs=4, evict="chain62",
                bc="pool", order="interleave2"):
    from contextlib import ExitStack

    import concourse.bacc as bacc
    import concourse.tile as tile
    from concourse import mybir

    f32 = mybir.dt.float32
    bf16 = mybir.dt.bfloat16
    u32 = mybir.dt.uint32

    nc = bacc.Bacc("TRN2", target_bir_lowering=False, debug=False)
    qt = nc.dram_tensor("qt", [KDIM, N], bf16, kind="ExternalInput")
    ct = nc.dram_tensor("ct", [KDIM, N], bf16, kind="ExternalInput")
    out_idx = nc.dram_tensor("out_idx", [QT, 128, 8], u32, kind="ExternalOutput")

    MAX = mybir.AluOpType.max

    with tile.TileContext(nc) as tc:
        with ExitStack() as ctx:
            const = ctx.enter_context(tc.tile_pool(name="const", bufs=1))
            big = ctx.enter_context(tc.tile_pool(name="big", bufs=big_bufs))
            small = ctx.enter_context(tc.tile_pool(name="small", bufs=small_bufs))
            psum = ctx.enter_context(
                tc.tile_pool(name="psum", bufs=psum_bufs, space="PSUM"))

            qt_s = const.tile([KDIM, N], bf16)
            nc.sync.dma_start(qt_s[:], qt.ap())
            ct_s = const.tile([KDIM, N], bf16)
            nc.sync.dma_start(ct_s[:], ct.ap())
            zeros8 = const.tile([128, 8], f32)
            nc.vector.memset(zeros8[:], 0.0)

            for t in range(QT):
                lhs = qt_s[:, t * 128:(t + 1) * 128]

                ev = nc.vector if evict == "dve" else nc.gpsimd
                # halve/pair-evicted groups: candidates [0, nh*1024)
                # ACT-evicted rest: casts of [nh*1024, 8192)
                nh = {"halves": 4, "halves5": 5, "dmah5": 5, "dmah4": 4,
                      "chain53": 0, "chain62": 0,
                      "halves45": 4 + (t % 2)}.get(evict, 4)
                tree = {"halves": "h4", "halves5": "h5", "dmah5": "h5",
                        "dmah4": "h4", "chain53": "c53", "chain62": "c62",
                        "halves45": "h4" if t % 2 == 0 else "h5"}.get(evict)
                if evict in ("chain53", "chain62"):
                    F = None
                    A = big.tile([128, 5120 if evict == "chain53" else 6144],
                                 bf16, tag="A")
                else:
                    F = big.tile(
                        [128, 512 * nh if evict.startswith("halves") else 2048],
                        bf16, tag="F")
                    A = big.tile([128, (8 - nh) * 1024], bf16, tag="A")

                def fill(base):
                    q0 = psum.tile([128, 1024], f32, tag="q", name="q0")
                    for j in range(2):
                        nc.tensor.matmul(
                            q0[:, j * 512:(j + 1) * 512],
                            lhs,
                            ct_s[:, base + j * 512: base + (j + 1) * 512],
                            start=True,
                            stop=True,
                        )
                    return q0

                def pair(g):
                    qp0 = fill((2 * g) * 1024)
                    qp1 = fill((2 * g + 1) * 1024)
                    ev.tensor_tensor(
                        out=F[:, g * 1024:(g + 1) * 1024],
                        in0=qp0[:], in1=qp1[:], op=MAX)

                def halve(g, engine):
                    # single-buf hold: fold one 1024-group to [512]
                    # out[i] = max(cand[1024g+i], cand[1024g+512+i])
                    qh = fill(g * 1024)
                    engine.tensor_tensor(
                        out=F[:, g * 512:(g + 1) * 512],
                        in0=qh[:, :512], in1=qh[:, 512:], op=MAX)

                def dma_halve(g):
                    # DMA stages PSUM->SBUF fp32 (GPSIMD can't touch PSUM);
                    # Pool halve-folds the staged copy
                    qh = fill(g * 1024)
                    S = small.tile([128, 1024], f32, tag=f"S{g}")
                    nc.sync.dma_start(S[:], qh[:])
                    nc.gpsimd.tensor_tensor(
                        out=F[:, g * 512:(g + 1) * 512],
                        in0=S[:, :512], in1=S[:, 512:], op=MAX)

                def cast(g):
                    qa = fill(nh * 1024 + g * 1024)
                    nc.scalar.activation(
                        A[:, g * 1024:(g + 1) * 1024],
                        qa[:],
                        mybir.ActivationFunctionType.Identity,
                        scale=1.0,
                    )

                if evict in ("chain53", "chain62"):
                    # ACT casts + DVE chain-TTs (evict+merge fused);
                    # chain seeded from A0 so chained groups need no merge
                    Es = []

                    def chain(g, seed):
                        qe = fill(g * 1024)
                        Ek = small.tile([128, 1024], bf16, tag=f"E{g}",
                                        name=f"E{g}")
                        nc.vector.tensor_tensor(
                            out=Ek[:], in0=qe[:], in1=seed, op=MAX)
                        Es.append(Ek)

                    if evict == "chain53":
                        cast(0); cast(1)
                        chain(5, A[:, :1024])
                        cast(2)
                        chain(6, Es[0][:])
                        cast(3)
                        chain(7, Es[1][:])
                        cast(4)
                    else:
                        cast(0); cast(1)
                        chain(6, A[:, :1024])
                        cast(2); cast(3)
                        chain(7, Es[0][:])
                        cast(4); cast(5)
                elif evict == "dmah5":
                    cast(0); cast(1); dma_halve(0); dma_halve(1)
                    cast(2); dma_halve(2); dma_halve(3); dma_halve(4)
                elif evict == "dmah4":
                    cast(0); cast(1); dma_halve(0); dma_halve(1)
                    cast(2); cast(3); dma_halve(2); dma_halve(3)
                elif tree == "h4":
                    hv = nc.gpsimd
                    cast(0); cast(1); halve(0, hv); halve(1, hv)
                    cast(2); cast(3); halve(2, hv); halve(3, hv)
                elif tree == "h5":
                    hv = nc.gpsimd
                    cast(0); cast(1); halve(0, hv); halve(1, hv)
                    cast(2); halve(2, hv); halve(3, hv); halve(4, hv)
                elif order == "pairs_first":
                    pair(0); pair(1); cast(0); cast(1); cast(2); cast(3)
                elif order == "acts_first":
                    cast(0); cast(1); cast(2); cast(3); pair(0); pair(1)
                elif order == "interleave":
                    pair(0); cast(0); cast(1); pair(1); cast(2); cast(3)
                elif order == "interleave2":
                    cast(0); cast(1); pair(0); cast(2); cast(3); pair(1)
                elif order == "rotate":
                    if t % 2 == 0:
                        cast(0); cast(1); pair(0); cast(2); cast(3); pair(1)
                    else:
                        pair(0); cast(0); cast(1); pair(1); cast(2); cast(3)
                elif order == "phase":
                    # interleave2 + a dummy q-ring alloc: shifts the psum ring
                    # phase by one slot per tile so pair holds rotate buffers
                    cast(0); cast(1); pair(0); cast(2); cast(3); pair(1)
                    dq = psum.tile([128, 8], f32, tag="q", name="dq")
                    nc.vector.memset(dq[:], 0.0)
                elif order == "interleave3":
                    # pairs 5 alloc-slots apart -> land on different buffers
                    cast(0); cast(1); pair(0); cast(2); pair(1); cast(3)
                elif order == "interleave3p":
                    cast(0); cast(1); pair(0)
                    dq = psum.tile([128, 8], f32, tag="q", name="dq")
                    nc.vector.memset(dq[:], 0.0)
                    cast(2); pair(1); cast(3)
                else:
                    raise ValueError(order)

                if tree == "c62":
                    # A[6144] (A0 seeded into chain) + E7 -> H3[256] on DVE
                    T1 = small.tile([128, 2048], bf16, tag="T1")
                    nc.vector.tensor_tensor(
                        out=T1[:], in0=A[:, 1024:3072], in1=A[:, 3072:5120],
                        op=MAX)
                    T2 = small.tile([128, 1024], bf16, tag="T2")
                    nc.vector.tensor_tensor(
                        out=T2[:], in0=A[:, 5120:6144], in1=Es[1][:], op=MAX)
                    T3 = small.tile([128, 1024], bf16, tag="T3")
                    nc.vector.tensor_tensor(
                        out=T3[:], in0=T1[:, :1024], in1=T1[:, 1024:], op=MAX)
                    T4 = small.tile([128, 1024], bf16, tag="T4")
                    nc.vector.tensor_tensor(out=T4[:], in0=T3[:], in1=T2[:],
                                            op=MAX)
                    T5 = small.tile([128, 512], bf16, tag="T5")
                    nc.vector.tensor_tensor(
                        out=T5[:], in0=T4[:, :512], in1=T4[:, 512:], op=MAX)
                    H3 = small.tile([128, 256], bf16, tag="H3")
                    nc.vector.tensor_tensor(
                        out=H3[:], in0=T5[:, :256], in1=T5[:, 256:], op=MAX)
                elif tree == "c53":
                    # Pool merges A1..A4; DVE joins the chain output E3
                    P1 = small.tile([128, 1024], bf16, tag="P1")
                    nc.gpsimd.tensor_tensor(
                        out=P1[:], in0=A[:, 1024:2048], in1=A[:, 2048:3072],
                        op=MAX)
                    P2 = small.tile([128, 1024], bf16, tag="P2")
                    nc.gpsimd.tensor_tensor(
                        out=P2[:], in0=A[:, 3072:4096], in1=A[:, 4096:5120],
                        op=MAX)
                    P3 = small.tile([128, 1024], bf16, tag="P3")
                    nc.gpsimd.tensor_tensor(out=P3[:], in0=P1[:], in1=P2[:],
                                            op=MAX)
                    W = small.tile([128, 1024], bf16, tag="W")
                    nc.vector.tensor_tensor(out=W[:], in0=Es[2][:], in1=P3[:],
                                            op=MAX)
                    f1 = small.tile([128, 512], bf16, tag="f1")
                    (nc.gpsimd if bc == "pool" else nc.vector).tensor_tensor(
                        out=f1[:], in0=W[:, :512], in1=W[:, 512:], op=MAX)
                    H3 = small.tile([128, 256], bf16, tag="H3")
                    nc.vector.tensor_tensor(
                        out=H3[:], in0=f1[:, :256], in1=f1[:, 256:], op=MAX)
                elif tree == "h5":
                    # A[3072] -> Zf[512]; F (5x[512] halve-folds) -> U[512]
                    X1 = small.tile([128, 1024], bf16, tag="X1")
                    nc.vector.tensor_tensor(
                        out=X1[:], in0=A[:, :1024], in1=A[:, 1024:2048], op=MAX)
                    X2 = small.tile([128, 1024], bf16, tag="X2")
                    nc.vector.tensor_tensor(
                        out=X2[:], in0=X1[:], in1=A[:, 2048:], op=MAX)
                    Zf = small.tile([128, 512], bf16, tag="Zf")
                    nc.vector.tensor_tensor(
                        out=Zf[:], in0=X2[:, :512], in1=X2[:, 512:], op=MAX)
                    p01 = small.tile([128, 512], bf16, tag="p01")
                    nc.vector.tensor_tensor(
                        out=p01[:], in0=F[:, :512], in1=F[:, 512:1024], op=MAX)
                    p23 = small.tile([128, 512], bf16, tag="p23")
                    nc.vector.tensor_tensor(
                        out=p23[:], in0=F[:, 1024:1536], in1=F[:, 1536:2048], op=MAX)
                    pf = small.tile([128, 512], bf16, tag="pf")
                    nc.vector.tensor_tensor(out=pf[:], in0=p01[:], in1=p23[:], op=MAX)
                    U = small.tile([128, 512], bf16, tag="U")
                    nc.vector.tensor_tensor(
                        out=U[:], in0=pf[:], in1=F[:, 2048:2560], op=MAX)
                    H2 = small.tile([128, 512], bf16, tag="H2")
                    nc.vector.tensor_tensor(out=H2[:], in0=U[:], in1=Zf[:], op=MAX)
                    H3 = small.tile([128, 256], bf16, tag="H3")
                    nc.vector.tensor_tensor(
                        out=H3[:], in0=H2[:, :256], in1=H2[:, 256:], op=MAX)
                elif tree == "h4":
                    # A[4096] -> Zf[512]; F (4x[512] halve-folds) -> pf[512]
                    PA = small.tile([128, 2048], bf16, tag="PA")
                    nc.vector.tensor_tensor(
                        out=PA[:], in0=A[:, :2048], in1=A[:, 2048:], op=MAX)
                    X = small.tile([128, 1024], bf16, tag="X")
                    nc.vector.tensor_tensor(
                        out=X[:], in0=PA[:, :1024], in1=PA[:, 1024:], op=MAX)
                    Zf = small.tile([128, 512], bf16, tag="Zf")
                    nc.vector.tensor_tensor(
                        out=Zf[:], in0=X[:, :512], in1=X[:, 512:], op=MAX)
                    p01 = small.tile([128, 512], bf16, tag="p01")
                    nc.vector.tensor_tensor(
                        out=p01[:], in0=F[:, :512], in1=F[:, 512:1024], op=MAX)
                    p23 = small.tile([128, 512], bf16, tag="p23")
                    nc.vector.tensor_tensor(
                        out=p23[:], in0=F[:, 1024:1536], in1=F[:, 1536:], op=MAX)
                    pf = small.tile([128, 512], bf16, tag="pf")
                    (nc.gpsimd if bc == "pool" else nc.vector).tensor_tensor(
                        out=pf[:], in0=p01[:], in1=p23[:], op=MAX)
                    H2 = small.tile([128, 512], bf16, tag="H2")
                    nc.vector.tensor_tensor(out=H2[:], in0=Zf[:], in1=pf[:], op=MAX)
                    H3 = small.tile([128, 256], bf16, tag="H3")
                    nc.vector.tensor_tensor(
                        out=H3[:], in0=H2[:, :256], in1=H2[:, 256:], op=MAX)
                elif evict == "pool":
                    # DVE owns every merge; Pool already did the evictions
                    PA = small.tile([128, 2048], bf16, tag="PA")
                    nc.vector.tensor_tensor(
                        out=PA[:], in0=A[:, :2048], in1=A[:, 2048:], op=MAX)
                    X = small.tile([128, 1024], bf16, tag="X")
                    nc.vector.tensor_tensor(
                        out=X[:], in0=PA[:, :1024], in1=PA[:, 1024:], op=MAX)
                    Y = small.tile([128, 1024], bf16, tag="Y")
                    nc.vector.tensor_tensor(
                        out=Y[:], in0=F[:, :1024], in1=F[:, 1024:], op=MAX)
                    H = small.tile([128, 1024], bf16, tag="H")
                    nc.vector.tensor_tensor(out=H[:], in0=X[:], in1=Y[:], op=MAX)
                else:
                    # Pool merges (gpsimd)
                    P1 = small.tile([128, 1024], bf16, tag="P1")
                    nc.gpsimd.tensor_tensor(
                        out=P1[:], in0=F[:, :1024], in1=F[:, 1024:], op=MAX)
                    P2 = small.tile([128, 1024], bf16, tag="P2")
                    nc.gpsimd.tensor_tensor(
                        out=P2[:], in0=A[:, :1024], in1=A[:, 1024:2048], op=MAX)
                    P3 = small.tile([128, 1024], bf16, tag="P3")
                    nc.gpsimd.tensor_tensor(
                        out=P3[:], in0=A[:, 2048:3072], in1=A[:, 3072:], op=MAX)
                    X = small.tile([128, 1024], bf16, tag="X")
                    nc.vector.tensor_tensor(out=X[:], in0=P1[:], in1=P2[:], op=MAX)
                    H = small.tile([128, 1024], bf16, tag="H")
                    nc.vector.tensor_tensor(out=H[:], in0=X[:], in1=P3[:], op=MAX)

                if not (evict.startswith("halves") or evict.startswith("chain")):
                    H2 = small.tile([128, 512], bf16, tag="H2")
                    nc.vector.tensor_tensor(
                        out=H2[:], in0=H[:, :512], in1=H[:, 512:], op=MAX)
                    H3 = small.tile([128, 256], bf16, tag="H3")
                    nc.vector.tensor_tensor(
                        out=H3[:], in0=H2[:, :256], in1=H2[:, 256:], op=MAX)

                smax = small.tile([128, 1], f32, tag="smax")
                nc.vector.tensor_scalar(
                    out=H3[:], in0=H3[:], scalar1=0.0, scalar2=None,
                    op0=mybir.AluOpType.add, op1=MAX, accum_out=smax[:])

                m8 = small.tile([128, 8], bf16, tag="m8")
                if bc == "pool":
                    nc.gpsimd.tensor_scalar(
                        out=m8[:], in0=zeros8[:], scalar1=smax[:], scalar2=None,
                        op0=mybir.AluOpType.add, op1=mybir.AluOpType.bypass)
                elif bc == "dve":
                    nc.vector.tensor_scalar(
                        out=m8[:], in0=zeros8[:], scalar1=smax[:], scalar2=None,
                        op0=mybir.AluOpType.add, op1=mybir.AluOpType.bypass)
                else:
                    nc.scalar.activation(
                        m8[:], zeros8[:],
                        mybir.ActivationFunctionType.Identity,
                        bias=smax[:], scale=0.0)

                idx8 = small.tile([128, 8], u32, tag="idx8")
                nc.vector.max_index(idx8[:], m8[:], H3[:])
                nc.sync.dma_start(out_idx.ap()[t], idx8[:])

    nc.compile()
    return nc


def _bf16_split3(x):
    import ml_dtypes

    bf = ml_dtypes.bfloat16
    hi = x.astype(bf)
    r1 = (x - hi.astype(np.float32)).astype(np.float32)
    lo = r1.astype(bf)
    r2 = (r1 - lo.astype(np.float32)).astype(np.float32)
    mid = r2.astype(bf)
    return hi, lo, mid


def _prep_core_inputs(q, c):
    """K=21 error-compensated bf16^3 packing of s = 2q.c - |c|^2."""
    import ml_dtypes

    bf = ml_dtypes.bfloat16
    qh, ql, qm = _bf16_split3(np.ascontiguousarray(q.T, np.float32))
    Ch, Cl, Cm = _bf16_split3(2.0 * np.ascontiguousarray(c.T, np.float32))
    csq = np.sum(c.astype(np.float32) * c.astype(np.float32), axis=1)
    sh, sl, sm = _bf16_split3(-csq)
    ones = np.ones((1, N), bf)
    qtm = np.concatenate(
        [qh, qh, ql, ql, qh, qm, ones, ones, ones], axis=0).astype(bf)
    ctm = np.concatenate(
        [Ch, Cl, Ch, Cl, Cm, Ch, sh[None], sl[None], sm[None]], axis=0).astype(bf)
    return {"qt": qtm, "ct": ctm}


def _d_row_fp32(q_row, c_all):
    return (
        np.sum(q_row * q_row).astype(np.float32)
        + np.sum(c_all * c_all, axis=1)
        - 2.0 * (c_all @ q_row)
    ).astype(np.float32)


def _indices_from_fold(idx8, q, c):
    """idx8: [QT, 128, 8] u32 folded positions (stride-MFOLD expansion)."""
    cand_f = idx8.reshape(N, 8)
    valid = cand_f != np.uint32(0xFFFFFFFF)
    ch = np.where(valid, cand_f, 0).astype(np.int64)
    cands = (ch[:, :, None]
             + (np.arange(EXPAND) * MFOLD)[None, None, :]).reshape(N, 8 * EXPAND)
    vmask = np.repeat(valid, EXPAND, axis=1)
    qf = q.astype(np.float32)
    cf = c.astype(np.float32)
    qsq = np.sum(qf * qf, axis=1).astype(np.float32)
    csq = np.sum(cf * cf, axis=1).astype(np.float32)
    dots = np.einsum("rkd,rd->rk", cf[cands], qf,
                     dtype=np.float32).astype(np.float32)
    dc = (qsq[:, None] + csq[cands] - np.float32(2.0) * dots).astype(np.float32)
    dc[~vmask] = np.inf
    best = np.argmin(dc, axis=1)
    idx = cands[np.arange(N), best]
    dmin = dc[np.arange(N), best]
    n_min = (dc == dmin[:, None]).sum(1)
    fix = np.where((n_min > 1) | valid[:, 7])[0]
    for r in fix:
        idx[r] = int(np.argmin(_d_row_fp32(qf[r], cf)))
    return idx


def _loss_one(q, c, idx):
    d = np.sum((q - c[idx]) ** 2, axis=1).astype(np.float32)
    cnt = np.bincount(idx, minlength=N).astype(np.float32)
    w = np.float32(1.0) / (cnt[idx] + np.float32(1e-6))
    return np.mean(np.float32(1.0) - np.exp(-d) * w, dtype=np.float32)


def run_cores(in_maps, trace=False):
    from concourse.bass_utils import run_bass_kernel_spmd

    if "nc" not in _CACHE:
        _CACHE["nc"] = _build_fold()
    nc = _CACHE["nc"]
    res = run_bass_kernel_spmd(
        nc, in_maps, core_ids=list(range(N_CORES)), trace=trace
    )
    return res


def kernel(gts, preds):
    gts = np.ascontiguousarray(np.asarray(gts, dtype=np.float32))
    preds = np.ascontiguousarray(np.asarray(preds, dtype=np.float32))

    qc = []
    for core in range(N_CORES):
        b, direction = core >> 1, core & 1
        if direction == 0:
            qc.append((gts[b], preds[b]))
        else:
            qc.append((preds[b], gts[b]))

    in_maps = [_prep_core_inputs(q, c) for (q, c) in qc]
    res = run_cores(in_maps)

    loss = np.zeros(B, np.float32)
    per_dir = {}
    for core in range(N_CORES):
        q, c = qc[core]
        idx = _indices_from_fold(np.asarray(res.results[core]["out_idx"]), q, c)
        per_dir[core] = _loss_one(q, c, idx)
    for b in range(B):
        loss[b] = (per_dir[2 * b] + per_dir[2 * b + 1]) / np.float32(2.0)
    return loss
